# revision 55
# baseline (speedup 1.0000x reference)
"""Causal multi-head attention on 8 Trainium2 NeuronCores.

Strategy: tensor-parallel over heads (16 heads / 8 cores = 2 heads per core).
Each core gets the full activations (as x^T) plus its column-shard of
Wq/Wk/Wv ([1024, 128]) and row-shard of Wo ([128, 1024]); the host sums the
8 partial outputs and adds bo. No device collectives.

Per-core pipeline:
  * Q/K/V projections run on the PE in fp8 DoubleRow mode with a hi+lo
    split (x = fp8(x) + fp8(x - fp8(x)), same for 32*W; the lo*lo term is
    dropped).  Three DoubleRow matmuls per 256-deep chunk cost 25% less
    than bf16 while being *more* accurate than bf16.
  * Scores are computed transposed (sc[k, q] = K^T.T @ Q^T) in bf16 with
    causally column-restricted matmuls; exp runs on ACT (scale=1/8 folded
    in), also column-restricted; the diagonal triangle is masked
    multiplicatively on the Pool engine.
  * A@V accumulates [V | 1] so row-sums fall out of the same PSUM tile,
    runs one group behind exp/mask so the PE never blocks on ACT, and the
    softmax division uses a DVE reciprocal + gpsimd partition broadcast +
    DVE multiply (gpsimd must not touch PSUM).  Head 1's slab is moved
    into the shared cc tile with a partition-shifting SBUF DMA.
  * The Wo matmul of call i is interleaved into the score/exp stream of
    call i+1 (software pipelining) so its PSUM->SBUF copies and DMAs hide.
  * Output partials are stored bf16.
"""

import os
import sys

for p in ("/opt/trn_rl_repo", "/root/.axon_site/_ro/trn_rl_repo"):
    if os.path.isdir(p) and p not in sys.path:
        sys.path.append(p)

import numpy as np
import ml_dtypes

import concourse.bass as bass
import concourse.bacc as bacc
import concourse.mybir as mybir
import concourse.tile as tile
from concourse.bass_utils import run_bass_kernel_spmd

BF16 = mybir.dt.bfloat16
FP8 = mybir.dt.float8e4
F32 = mybir.dt.float32
NP_BF16 = ml_dtypes.bfloat16
NP_FP8 = ml_dtypes.float8_e4m3

D_MODEL = 1024
NUM_HEADS = 16
D_K = 64
B = 2
S = 2048
T = B * S            # 4096 tokens
N_CORES = 8
FPC = 128            # features per core (2 heads x 64)

_AluOp = mybir.AluOpType
_Act = mybir.ActivationFunctionType
_DR = mybir.MatmulPerfMode.DoubleRow


def build_nc():
    nc = bacc.Bacc()

    # hi/lo split fp8 activations, pair-packed for DoubleRow:
    # x*[p, j, i, t] = xT[256j + 128i + p, t]
    xh = nc.declare_dram_parameter("xh", [128, 4, 2, T], FP8, isOutput=False)
    xl = nc.declare_dram_parameter("xl", [128, 4, 2, T], FP8, isOutput=False)
    wqh = nc.declare_dram_parameter("wqh", [128, 4, 2, FPC], FP8, isOutput=False)
    wql = nc.declare_dram_parameter("wql", [128, 4, 2, FPC], FP8, isOutput=False)
    wkh = nc.declare_dram_parameter("wkh", [128, 4, 2, FPC], FP8, isOutput=False)
    wkl = nc.declare_dram_parameter("wkl", [128, 4, 2, FPC], FP8, isOutput=False)
    wvh = nc.declare_dram_parameter("wvh", [128, 4, 2, FPC], FP8, isOutput=False)
    wvl = nc.declare_dram_parameter("wvl", [128, 4, 2, FPC], FP8, isOutput=False)
    wo = nc.declare_dram_parameter("wo", [FPC, D_MODEL], BF16, isOutput=False)
    bq = nc.declare_dram_parameter("bq", [FPC, 512], F32, isOutput=False)
    bk = nc.declare_dram_parameter("bk", [FPC, 512], F32, isOutput=False)
    bv = nc.declare_dram_parameter("bv", [1, FPC], F32, isOutput=False)
    tri = nc.declare_dram_parameter("tri", [128, 128], BF16, isOutput=False)
    out = nc.declare_dram_parameter("out", [T, D_MODEL], BF16, isOutput=True)

    with tile.TileContext(nc) as tc:
        with (
            tc.tile_pool(name="persist", bufs=1) as persist,
            tc.tile_pool(name="at_pool", bufs=6) as at_pool,
            tc.tile_pool(name="cc_pool", bufs=2) as cc_pool,
            tc.tile_pool(name="h1_pool", bufs=3) as h1_pool,
            tc.tile_pool(name="rt_pool", bufs=2) as rt_pool,
            tc.tile_pool(name="rb_pool", bufs=2) as rb_pool,
            tc.tile_pool(name="ob_pool", bufs=8) as ob_pool,
        ):
            # ---------- persistent loads ----------
            # HWDGE descriptor generation serializes (~625ns each), so the
            # issue order below is the arrival order: batch-0 activations
            # and the Q/K hi weights come first.
            def loadw(dram, tag):
                # SWDGE path (gpsimd): stays off the serialized HWDGE
                # descriptor queue that the x-tile loads occupy
                t_ = persist.tile([128, 4, 2, FPC], FP8, tag=tag, name=tag)
                nc.gpsimd.dma_start(out=t_, in_=dram[:, :, :, :])
                return t_

            xth = [[None] * B for _ in range(4)]
            xtl = [[None] * B for _ in range(4)]

            def load_x(kind, j, bb, q0, q1):
                """DMA token range [q0*512, q1*512) of an x tile; split loads
                let the first projections start before full tiles land."""
                dram, arr = (xh, xth) if kind == "h" else (xl, xtl)
                t_ = arr[j][bb]
                nc.sync.dma_start(
                    out=t_[:, :, q0 * 512:q1 * 512],
                    in_=dram[:, j, :, bb * S + q0 * 512:bb * S + q1 * 512])

            for j in range(4):
                for bb in range(B):
                    xth[j][bb] = persist.tile(
                        [128, 2, S], FP8, tag=f"xh{j}_{bb}", name=f"xh{j}_{bb}")
                    xtl[j][bb] = persist.tile(
                        [128, 2, S], FP8, tag=f"xl{j}_{bb}", name=f"xl{j}_{bb}")

            wqh_sb = loadw(wqh, "wqh")
            wql_sb = loadw(wql, "wql")
            wkh_sb = loadw(wkh, "wkh")
            wkl_sb = loadw(wkl, "wkl")
            wvh_sb = loadw(wvh, "wvh")
            wvl_sb = loadw(wvl, "wvl")
            bqs = persist.tile([128, 512], F32, tag="bq")
            nc.sync.dma_start(out=bqs, in_=bq[:, :])
            bks = persist.tile([128, 512], F32, tag="bk")
            nc.sync.dma_start(out=bks, in_=bk[:, :])
            bvs = persist.tile([128, FPC], F32, tag="bv")
            nc.gpsimd.dma_start(out=bvs, in_=bv.ap().to_broadcast([128, FPC]))
            tri_sb = persist.tile([128, 128], BF16, tag="tri")
            nc.gpsimd.dma_start(out=tri_sb, in_=tri[:, :])

            # batch-0: token-quarters for the first half (fast start),
            # then one half-tile load; batch-1 is issued later, between
            # attention calls, to keep the HWDGE descriptor queue short
            for q in range(2):
                for j in range(4):
                    load_x("h", j, 0, q, q + 1)
                    load_x("l", j, 0, q, q + 1)
            for j in range(4):
                load_x("h", j, 0, 2, 4)
                load_x("l", j, 0, 2, 4)
            wo_sb = persist.tile([FPC, D_MODEL], BF16, tag="wo")
            nc.sync.dma_start(out=wo_sb, in_=wo[:, :])

            qt_tiles = [persist.tile([128, 512], BF16, tag=f"qt{i}",
                                     name=f"qt{i}") for i in range(T // 512)]
            kt_tiles = [persist.tile([128, 512], BF16, tag=f"kt{i}",
                                     name=f"kt{i}") for i in range(T // 512)]
            # V: per 128-token k-tile, [V_h0 | 1 | V_h1 | 1] (ones at 64/129)
            v_sb = [persist.tile([128, 130], BF16, tag=f"v{g}", name=f"v{g}")
                    for g in range(T // 128)]
            for g in range(T // 128):
                nc.vector.memset(v_sb[g][:, 64:65], 1.0)
                nc.vector.memset(v_sb[g][:, 129:130], 1.0)

            with (
                tc.tile_pool(name="big", bufs=3, space="PSUM") as big,
                tc.tile_pool(name="avp", bufs=2, space="PSUM") as avp,
            ):
                def dr_proj(dst_ap, whi, wlo, bb, sl, n):
                    """12 DoubleRow matmuls accumulating x@ (32W) into psum;
                    grouped by contraction pair j to track DMA arrival."""
                    mms = []
                    for j in range(4):
                        mms.append((whi[:, j], xth[j][bb][:, :, sl]))
                        mms.append((wlo[:, j], xth[j][bb][:, :, sl]))
                        mms.append((whi[:, j], xtl[j][bb][:, :, sl]))
                    for i, (lhs, rhs) in enumerate(mms):
                        nc.tensor.matmul(
                            dst_ap, lhsT=lhs, rhs=rhs,
                            start=(i == 0), stop=(i == len(mms) - 1),
                            perf_mode=_DR,
                        )

                def proj_q(ti):
                    bb, loc = ti // 4, (ti % 4) * 512
                    p2 = big.tile([128, 2, 512], F32, tag="big", name="pq")
                    dr_proj(p2[:, 0, :], wqh_sb, wql_sb, bb,
                            slice(loc, loc + 512), 512)
                    nc.vector.scalar_tensor_tensor(
                        out=qt_tiles[ti], in0=p2[:, 0, :], scalar=1.0 / 32.0,
                        in1=bqs, op0=_AluOp.mult, op1=_AluOp.add)

                def proj_k(ti):
                    bb, loc = ti // 4, (ti % 4) * 512
                    p2 = big.tile([128, 2, 512], F32, tag="big", name="pk")
                    dr_proj(p2[:, 0, :], wkh_sb, wkl_sb, bb,
                            slice(loc, loc + 512), 512)
                    nc.vector.scalar_tensor_tensor(
                        out=kt_tiles[ti], in0=p2[:, 0, :], scalar=1.0 / 32.0,
                        in1=bks, op0=_AluOp.mult, op1=_AluOp.add)

                def proj_v(g):
                    bb, loc = g // (S // 128), (g % (S // 128)) * 128
                    gsl = slice(loc, loc + 128)
                    p2 = big.tile([128, 2, 512], F32, tag="big", name="pv")
                    pv = p2[:, 0, 0:FPC]
                    mms = []
                    for j in range(4):
                        mms.append((xth[j][bb][:, :, gsl], wvh_sb[:, j]))
                        mms.append((xth[j][bb][:, :, gsl], wvl_sb[:, j]))
                        mms.append((xtl[j][bb][:, :, gsl], wvh_sb[:, j]))
                    for i, (lhs, rhs) in enumerate(mms):
                        nc.tensor.matmul(
                            pv, lhsT=lhs, rhs=rhs,
                            start=(i == 0), stop=(i == len(mms) - 1),
                            perf_mode=_DR,
                        )
                    for h in range(2):
                        nc.vector.scalar_tensor_tensor(
                            out=v_sb[g][:, 65 * h:65 * h + 64],
                            in0=p2[:, 0, h * 64:h * 64 + 64],
                            scalar=1.0 / 32.0,
                            in1=bvs[:, h * 64:h * 64 + 64],
                            op0=_AluOp.mult, op1=_AluOp.add)

                # engine rotation for output copies: ACT/DVE/Pool round-robin
                cp_state = [0]

                def copy_out(ob, p2):
                    # only DVE and ACT may read PSUM; ACT joins once its exp
                    # backlog drains in the endgame
                    i = cp_state[0]
                    cp_state[0] += 1
                    if i < 20:
                        nc.vector.tensor_copy(ob, p2)
                    elif i % 3 == 2:
                        nc.vector.tensor_copy(ob, p2)
                    else:
                        nc.scalar.copy(ob, p2)

                def mask_mul(ap, m):
                    # gpsimd: at/tri are SBUF (gpsimd must not touch PSUM)
                    nc.gpsimd.tensor_mul(ap, ap, m)

                def attention_compute(b, qt, fillers):
                    """Scores/exp/AV + softmax normalize; returns cc tile.

                    Head 1 runs first so its partition-shifting SBUF DMA into
                    cc[64:128] hides under head 0's score/exp stream.  The
                    score matmuls run one group ahead of exp/AV so the
                    in-order PE queue never blocks on the ACT engine."""
                    cc = cc_pool.tile([128, 512], BF16, tag="cc", name="cc")
                    nk = (qt + 1) * 4
                    ng = nk // 2

                    def emit_sc(h, grp):
                        hsl = slice(h * 64, h * 64 + 64)
                        k0 = 2 * grp
                        c0 = max(0, (k0 - 4 * qt) * 128)
                        sc = big.tile([128, 2, 512], F32, tag="big", name="sc")
                        for j in range(2):
                            ki = k0 + j
                            kt_i = b * 4 + ki // 4
                            ko = (ki % 4) * 128
                            nc.tensor.matmul(
                                sc[:, j, c0:512],
                                lhsT=kt_tiles[kt_i][hsl, ko:ko + 128],
                                rhs=qt_tiles[b * 4 + qt][hsl, c0:512],
                                start=True, stop=True,
                            )
                        return sc, c0

                    def emit_av(h, av, grp, at):
                        for j in range(2):
                            ki = 2 * grp + j
                            cj = max(0, (ki - 4 * qt) * 128)
                            g = b * (S // 128) + ki
                            nc.tensor.matmul(
                                av[:, cj:512],
                                lhsT=v_sb[g][:, 65 * h:65 * h + 65],
                                rhs=at[:, j, cj:512],
                                start=(ki == 0), stop=(ki == nk - 1),
                            )

                    sc_q = [emit_sc(1, 0), emit_sc(1, 1)]
                    for h in (1, 0):
                        av = avp.tile([65, 512], F32, tag="av", name="av")
                        pend = None
                        for grp in range(ng):
                            sc, c0 = sc_q.pop(0)
                            k0 = 2 * grp
                            at = at_pool.tile([128, 2, 512], BF16, tag="at",
                                              name="at")
                            nc.scalar.activation(
                                out=at[:, :, c0:512], in_=sc[:, :, c0:512],
                                func=_Act.Exp, scale=0.125)
                            for j in range(2):
                                rel = k0 + j - 4 * qt
                                if rel >= 0:
                                    m0 = rel * 128
                                    mask_mul(at[:, j, m0:m0 + 128], tri_sb)
                            # AV runs one group late so its exp+mask inputs
                            # are always complete when the PE reaches it
                            if pend is not None:
                                emit_av(h, av, *pend)
                            pend = (grp, at)
                            if fillers:
                                f = fillers.pop(0)
                                if f is not None:
                                    f()
                            nxt = grp + 2
                            if nxt < ng:
                                sc_q.append(emit_sc(h, nxt))
                            elif h == 1:
                                sc_q.append(emit_sc(0, nxt - ng))
                        emit_av(h, av, *pend)
                        rt = rt_pool.tile([1, 512], BF16, tag="rt", name="rt")
                        with nc.allow_low_precision(
                                reason="bf16 softmax reciprocal"):
                            nc.vector.reciprocal(rt, av[64:65, :])
                        rb = rb_pool.tile([64, 512], BF16, tag="rb", name="rb")
                        nc.gpsimd.partition_broadcast(rb, rt)
                        if h == 0:
                            nc.vector.tensor_mul(cc[0:64, :], av[0:64, :], rb)
                        else:
                            h1t = h1_pool.tile([64, 512], BF16, tag="h1",
                                               name="h1t")
                            nc.vector.tensor_mul(h1t, av[0:64, :], rb)
                            nc.sync.dma_start(out=cc[64:128, :], in_=h1t)
                    return cc

                def wo_unit(b, qt, cc, ot):
                    def run():
                        tok0 = b * S + qt * 512
                        p2 = big.tile([128, 2, 512], F32, tag="big", name="po")
                        for n2 in range(2):
                            nc.tensor.matmul(
                                p2[:, n2, :],
                                lhsT=cc[:, ot * 128:(ot + 1) * 128],
                                rhs=wo_sb[:, n2 * 512:(n2 + 1) * 512],
                                start=True, stop=True,
                            )
                        r0 = tok0 + ot * 128
                        ob = ob_pool.tile([128, 2, 512], BF16, tag="ob",
                                          name="ob")
                        copy_out(ob, p2)
                        nc.sync.dma_start(out=out[r0:r0 + 128, :], in_=ob)
                    return run

                # ---------- schedule ----------
                # queue of (wo_seq or None, min_slot, fn); wo units from call
                # seq-1 may not occupy the first two filler slots of call seq
                # (their cc is still in flight there), and units may declare
                # a minimum slot (used to hold batch-1 projections until
                # their x tiles clear the DMA pipe)
                queue = []
                call_seq = [0]

                def attn(b, qt):
                    seq = call_seq[0]
                    call_seq[0] += 1
                    nslots = (qt + 1) * 4
                    take = []
                    while len(take) < nslots:
                        slot = len(take)
                        found = None
                        for idx, (tseq, mslot, _fn) in enumerate(queue):
                            if slot < 4 and tseq == seq - 1:
                                continue
                            if slot < mslot:
                                continue
                            found = idx
                            break
                        if found is None:
                            if queue:
                                take.append(None)
                                continue
                            break
                        take.append(queue.pop(found)[2])
                    cc = attention_compute(b, qt, take)
                    for ot in range(4):
                        queue.append((seq, 2, wo_unit(b, qt, cc, ot)))

                # batch-0 projections interleaved with its attention; the
                # ascending-qt order feeds ACT a growing exp backlog while
                # the PE alternates projection blocks and score streams.
                proj_q(0); proj_k(0)
                proj_v(0); proj_v(1)
                # v2/v3 are consumed only by attn(0,0)'s flushed last AV
                # group, so they fit in its filler slots
                queue.append((None, 0, lambda: proj_v(2)))
                queue.append((None, 0, lambda: proj_v(3)))
                attn(0, 0)
                # batch-1 activations: whole tiles, one descriptor each;
                # issued early so they clear the pipe by ~t27
                for j in range(4):
                    load_x("h", j, 1, 0, 4)
                    load_x("l", j, 1, 0, 4)
                proj_q(1); proj_k(1)
                # v4-v7 land in attn(0,1)'s empty early slots: first consumer
                # is the one-group-late AV of group 2 (slot 3 at the latest)
                for g in range(4, 8):
                    queue.append((None, 0, lambda g=g: proj_v(g)))
                attn(0, 1)
                # (0,2)'s projections run in attn(0,3)'s FIRST filler
                # slots: attn(0,3) itself consumes kt2/qt2 (score groups 4+)
                # and v8-v11 (AV groups 4,5), and the score stream runs one
                # group ahead, so these must land by slots 0-3.
                queue.append((None, 0, lambda: proj_q(2)))
                queue.append((None, 0, lambda: proj_k(2)))
                queue.append((None, 0, lambda: proj_v(8)))
                queue.append((None, 0, lambda: proj_v(9)))
                # batch-1 units parked until their x tiles clear the DMA pipe
                for ti in range(4, 8):
                    queue.append((None, 5, lambda ti=ti: proj_q(ti)))
                    queue.append((None, 5, lambda ti=ti: proj_k(ti)))
                for g in range(16, 32):
                    queue.append((None, 5, lambda g=g: proj_v(g)))
                # biggest batch-0 call third, smaller (0,2) last: batch-1
                # projections drain inside it with no ACT bubble
                proj_q(3); proj_k(3)
                for g in range(12, 16):
                    proj_v(g)
                proj_v(10); proj_v(11)
                attn(0, 3)
                # batch-1 x is in SBUF by now; lift the slot parking
                queue[:] = [(t, 0, f) for (t, _m, f) in queue]
                attn(0, 2)
                # drain any batch-1 projections that didn't fit in the slots
                rest = [fn for tseq, _ms, fn in queue if tseq is None]
                queue[:] = [e for e in queue if e[0] is not None]
                for fn in rest:
                    fn()
                for qt in [3, 2, 1, 0]:
                    attn(1, qt)
                for _tseq, _ms, fn in queue:
                    fn()
                queue.clear()
    return nc


_NC_CACHE = None


def _get_nc():
    global _NC_CACHE
    if _NC_CACHE is None:
        _NC_CACHE = build_nc()
        if not _NC_CACHE.is_finalized():
            _NC_CACHE.finalize()
    return _NC_CACHE


def _split8(a):
    """hi/lo fp8 split: a ~ hi + lo with hi = fp8(a)."""
    hi = a.astype(NP_FP8)
    lo = (a - hi.astype(np.float32)).astype(NP_FP8)
    return hi, lo


def _pack_x(xq):
    """[D, T] -> [128, 4, 2, T] with x[p,j,i,t] = xT[256j+128i+p, t]."""
    return np.ascontiguousarray(
        xq.reshape(4, 2, 128, xq.shape[1]).transpose(2, 0, 1, 3))


def _pack_w(w):
    """[D, FPC] -> [128, 4, 2, FPC]."""
    return np.ascontiguousarray(
        w.reshape(4, 2, 128, w.shape[1]).transpose(2, 0, 1, 3))


def _shard_inputs(x, Wq, bq, Wk, bk, Wv, bv, Wo, bo):
    x = np.asarray(x, np.float32)
    Wq, Wk, Wv, Wo = (np.asarray(a, np.float32) for a in (Wq, Wk, Wv, Wo))
    bq, bk, bv = (np.asarray(a, np.float32) for a in (bq, bk, bv))

    xT = np.ascontiguousarray(x.reshape(T, D_MODEL).T)
    xh_, xl_ = _split8(xT)
    xh_p = _pack_x(xh_)
    xl_p = _pack_x(xl_)

    p = np.arange(128)[:, None]
    f = np.arange(128)[None, :]
    tri = (p <= f).astype(NP_BF16)

    in_maps = []
    for c in range(N_CORES):
        fs = slice(c * FPC, (c + 1) * FPC)
        wqh_, wql_ = _split8(Wq[:, fs] * 32.0)
        wkh_, wkl_ = _split8(Wk[:, fs] * 32.0)
        wvh_, wvl_ = _split8(Wv[:, fs] * 32.0)
        in_maps.append({
            "xh": xh_p,
            "xl": xl_p,
            "wqh": _pack_w(wqh_), "wql": _pack_w(wql_),
            "wkh": _pack_w(wkh_), "wkl": _pack_w(wkl_),
            "wvh": _pack_w(wvh_), "wvl": _pack_w(wvl_),
            "wo": np.ascontiguousarray(Wo[fs, :]).astype(NP_BF16),
            "bq": np.ascontiguousarray(
                np.broadcast_to(bq[fs][:, None], (FPC, 512))).astype(np.float32),
            "bk": np.ascontiguousarray(
                np.broadcast_to(bk[fs][:, None], (FPC, 512))).astype(np.float32),
            "bv": np.ascontiguousarray(bv[fs]).reshape(1, FPC).astype(np.float32),
            "tri": tri,
        })
    return in_maps


def _gather(results, bo):
    total = np.zeros((T, D_MODEL), np.float32)
    for c in range(N_CORES):
        total += np.asarray(results[c]["out"], np.float32)
    total += np.asarray(bo, np.float32)[None, :]
    return total.reshape(B, S, D_MODEL)


def kernel(x, Wq, bq, Wk, bk, Wv, bv, Wo, bo):
    in_maps = _shard_inputs(x, Wq, bq, Wk, bk, Wv, bv, Wo, bo)
    nc = _get_nc()
    res = run_bass_kernel_spmd(nc, in_maps, list(range(N_CORES)))
    return _gather(res.results, bo)


if __name__ == "__main__":
    rng = np.random.default_rng(0)
    x = rng.standard_normal((B, S, D_MODEL)).astype(np.float32)
    sc = 1 / np.sqrt(D_MODEL)
    args = dict(
        x=x,
        Wq=rng.standard_normal((D_MODEL, D_MODEL)).astype(np.float32) * sc,
        bq=np.zeros(D_MODEL, np.float32),
        Wk=rng.standard_normal((D_MODEL, D_MODEL)).astype(np.float32) * sc,
        bk=np.zeros(D_MODEL, np.float32),
        Wv=rng.standard_normal((D_MODEL, D_MODEL)).astype(np.float32) * sc,
        bv=np.zeros(D_MODEL, np.float32),
        Wo=rng.standard_normal((D_MODEL, D_MODEL)).astype(np.float32) * sc,
        bo=np.zeros(D_MODEL, np.float32),
    )
    out = kernel(**args)
    print("kernel output", out.shape, out.dtype, np.abs(out).max())
